# revision 1
# baseline (speedup 1.0000x reference)
"""Trainium2 Bass kernel for nn_Conv2d_20590073217670.

Conv2d: input [32,64,64,64] (NCHW), weight [576,128] (unfold layout:
row = ci*9 + a*3 + b for tap (a,b)), bias [1,128,1,1], stride 1, pad 1.
Output [32,128,64,64].

Strategy: data-parallel over batch — 4 images per NeuronCore, 8 cores.
Per image, implicit GEMM: out[co, y, x] = sum_{a,b,ci} W[ci,a,b,co] *
img[ci, y+a-1, x+b-1].  SBUF holds the image on partitions 0:64 and a
one-row-down shifted copy on partitions 64:128, so a single K=128
matmul accumulates two vertical taps (a, a+1) at once.  The rounded
fp32r image is stored column-padded ([128, 64, 66], zero borders), so
every matmul is a full 64-wide slide satisfying the fp32r ISA
restrictions (even innermost count, 8B-aligned full-bank PSUM output).
Row borders are handled by restricting output rows (PSUM has_written
zero-fill keeps partial accumulation exact).  DVE produces all matmul
inputs (fp32->fp32r rounding) and evicts PSUM with a fused bias add.
"""
import sys

for _p in ("/opt/trn_rl_repo", "/root/.axon_site/_ro/trn_rl_repo"):
    if _p not in sys.path:
        sys.path.append(_p)

import numpy as np
from contextlib import ExitStack

import concourse.bacc as bacc
import concourse.tile as tile
from concourse import mybir
from concourse.bass_utils import run_bass_kernel_spmd

f32 = mybir.dt.float32
f32r = mybir.dt.float32r

N_CORES = 8
NB = 4  # images per core


def build_nc():
    nc = bacc.Bacc()
    x = nc.declare_dram_parameter("x", [NB, 64, 64, 64], f32, isOutput=False)
    w = nc.declare_dram_parameter("w", [576, 128], f32, isOutput=False)
    bias = nc.declare_dram_parameter("b", [128, 1], f32, isOutput=False)
    out = nc.declare_dram_parameter("out", [NB, 128, 64, 64], f32, isOutput=True)

    with tile.TileContext(nc) as tc, ExitStack() as ctx:
        const = ctx.enter_context(tc.tile_pool(name="const", bufs=1))
        xs_pool = ctx.enter_context(tc.tile_pool(name="xs", bufs=3))
        xr_pool = ctx.enter_context(tc.tile_pool(name="xr", bufs=3))
        ob_pool = ctx.enter_context(tc.tile_pool(name="ob", bufs=2))
        ps_pool = ctx.enter_context(tc.tile_pool(name="ps", bufs=8, space="PSUM"))

        # ---- weights: one [128, 9, 128] tile; partition p<64 holds channel
        # p's taps 0..8, partition 64+ci holds channel ci's taps 3..8 at
        # slots 0..5 (tap axis pre-shifted by -3).  Then the lhsT view
        # wr[:, t, :] pairs taps (t, t+3) across the partition halves:
        #   t in 0..2  -> taps (0,b) & (1,b)
        #   t in 3..5  -> taps (1,b) & (2,b)
        w3 = w[:].rearrange("(c t) m -> c t m", t=9)
        ws = const.tile([128, 9, 128], f32)
        wr = const.tile([128, 9, 128], f32r)
        bt = const.tile([128, 1], f32)
        zc = const.tile([128, 64, 1], f32)
        nc.sync.dma_start(out=ws[0:64, :, :], in_=w3)
        nc.sync.dma_start(out=ws[64:128, 0:6, :], in_=w3[:, 3:9, :])
        nc.sync.dma_start(out=bt[:], in_=bias[:])
        nc.vector.memset(zc[:], 0.0)
        nc.vector.tensor_copy(wr[0:64, :, :], ws[0:64, :, :])
        nc.vector.tensor_copy(wr[64:128, 0:6, :], ws[64:128, 0:6, :])

        for n in range(NB):
            xs = xs_pool.tile([128, 64, 64], f32)
            xr = xr_pool.tile([128, 64, 66], f32r)
            # image rows on partitions 0:64; one-row-down copy on 64:128
            nc.sync.dma_start(out=xs[0:64, :, :], in_=x[n])
            nc.sync.dma_start(out=xs[64:128, 0:63, :], in_=xs[0:64, 1:64, :])
            # fp32 -> fp32r rounding (DVE) into the column-padded layout;
            # upper-half row 63 is never read.  Zero border columns.
            nc.vector.tensor_copy(xr[:, 0:63, 1:65], xs[:, 0:63, :])
            nc.vector.tensor_copy(xr[0:64, 63, 1:65], xs[0:64, 63, :])
            nc.vector.tensor_copy(xr[:, :, 0:1], zc[:])
            nc.vector.tensor_copy(xr[:, :, 65:66], zc[:])

            osb = ob_pool.tile([128, 64, 64], f32)
            for blk in range(8):
                y0 = blk * 8
                P = ps_pool.tile([128, 8, 64], f32)
                if blk == 0:
                    pair_t, pr0 = 3, 0      # taps (1,2), rhs rows y0..y0+7
                else:
                    pair_t, pr0 = 0, y0 - 1  # taps (0,1), rhs rows y0-1..y0+6
                # b=1 first: full [8,64] coverage zero-fills the whole bank
                for k, b in enumerate((1, 0, 2)):
                    nc.tensor.matmul(
                        P[:, 0:8, :],
                        wr[:, pair_t + b, :],
                        xr[:, pr0:pr0 + 8, b:b + 64],
                        start=(k == 0), stop=False,
                    )
                # remaining vertical tap as K=64 single on partitions 0:64
                for k, b in enumerate((1, 0, 2)):
                    last = k == 2
                    if blk == 0:
                        # tap (0,b): out rows 1..7 read img rows 0..6
                        nc.tensor.matmul(
                            P[:, 1:8, :], wr[0:64, b, :],
                            xr[0:64, 0:7, b:b + 64],
                            start=False, stop=last,
                        )
                    elif blk == 7:
                        # tap (2,b): out rows 56..62 read img rows 57..63
                        nc.tensor.matmul(
                            P[:, 0:7, :], wr[0:64, 6 + b, :],
                            xr[0:64, 57:64, b:b + 64],
                            start=False, stop=last,
                        )
                    else:
                        nc.tensor.matmul(
                            P[:, 0:8, :], wr[0:64, 6 + b, :],
                            xr[0:64, y0 + 1:y0 + 9, b:b + 64],
                            start=False, stop=last,
                        )
                nc.vector.tensor_scalar_add(osb[:, y0:y0 + 8, :], P[:, :, :], bt[:])

            nc.sync.dma_start(out=out[n], in_=osb[:])

    nc.finalize()
    return nc


_NC = None


def _get_nc():
    global _NC
    if _NC is None:
        _NC = build_nc()
    return _NC


def kernel(**inputs) -> np.ndarray:
    x = np.ascontiguousarray(np.asarray(inputs["input"], dtype=np.float32))
    w = np.ascontiguousarray(np.asarray(inputs["weight"], dtype=np.float32))
    b = np.ascontiguousarray(
        np.asarray(inputs["bias"], dtype=np.float32).reshape(128, 1))
    nc = _get_nc()
    in_maps = [
        {"x": x[c * NB:(c + 1) * NB], "w": w, "b": b} for c in range(N_CORES)
    ]
    res = run_bass_kernel_spmd(nc, in_maps, list(range(N_CORES)))
    return np.concatenate([r["out"] for r in res.results], axis=0)



# revision 2
# speedup vs baseline: 1.1682x; 1.1682x over previous
"""Trainium2 Bass kernel for nn_Conv2d_20590073217670.

Conv2d: input [32,64,64,64] (NCHW), weight [576,128] (unfold layout:
row = ci*9 + a*3 + b for tap (a,b)), bias [1,128,1,1], stride 1, pad 1.
Output [32,128,64,64].

Strategy: data-parallel over batch — 4 images per NeuronCore, 8 cores.
Per image, implicit GEMM in bf16: out[co, y, x] = sum_{a,b,ci}
W[ci,a,b,co] * img[ci, y+a-1, x+b-1].  The image is cast fp32->bf16
into a fully zero-padded [128, 66, 66] SBUF tile: partitions 0:64 hold
img[c, r-1, j-1] (one-row/one-col border of zeros on every side),
partitions 64:128 hold the same data shifted up one row.  Every matmul
is then a uniform full [8, 64] output block — border taps read the
zero padding, so no per-block edge restrictions are needed.  Per
8-row block: 3 K=128 matmuls (vertical tap pairs (0,b)+(1,b)) plus
3 K=64 matmuls (taps (2,b)) = 6 passes.  bf16 halves the PE's moving-
operand SBUF traffic vs fp32r and enables the automatic fast-weight-
load path.  Input casts alternate between the DVE and Activation
engines per image; PSUM eviction (fused bias add) alternates between
them per block.
"""
import sys

for _p in ("/opt/trn_rl_repo", "/root/.axon_site/_ro/trn_rl_repo"):
    if _p not in sys.path:
        sys.path.append(_p)

import numpy as np
from contextlib import ExitStack

import concourse.bacc as bacc
import concourse.tile as tile
from concourse import mybir
from concourse.bass_utils import run_bass_kernel_spmd

f32 = mybir.dt.float32
bf16 = mybir.dt.bfloat16

N_CORES = 8
NB = 4  # images per core


def build_nc():
    nc = bacc.Bacc()
    x = nc.declare_dram_parameter("x", [NB, 64, 64, 64], f32, isOutput=False)
    w = nc.declare_dram_parameter("w", [576, 128], f32, isOutput=False)
    bias = nc.declare_dram_parameter("b", [128, 1], f32, isOutput=False)
    out = nc.declare_dram_parameter("out", [NB, 128, 64, 64], f32, isOutput=True)

    with tile.TileContext(nc) as tc, ExitStack() as ctx:
        const = ctx.enter_context(tc.tile_pool(name="const", bufs=1))
        xf_pool = ctx.enter_context(tc.tile_pool(name="xf", bufs=2))
        xb_pool = ctx.enter_context(tc.tile_pool(name="xb", bufs=2))
        ob_pool = ctx.enter_context(tc.tile_pool(name="ob", bufs=2))
        ps_pool = ctx.enter_context(tc.tile_pool(name="ps", bufs=8, space="PSUM"))

        # ---- weights [128, 9, 128] bf16: partition p<64 holds channel p's
        # taps 0..8; partition 64+ci holds channel ci's taps 3..8 at slots
        # 0..5 (tap axis pre-shifted by -3).  So wb[:, b, :] pairs taps
        # (0,b) lower / (1,b) upper, and wb[64:128, 3+b, :] is tap (2,b).
        w3 = w[:].rearrange("(c t) m -> c t m", t=9)
        ws = const.tile([128, 9, 128], f32)
        wb = const.tile([128, 9, 128], bf16)
        bt = const.tile([128, 1], f32)
        nc.sync.dma_start(out=ws[0:64, :, :], in_=w3)
        nc.sync.dma_start(out=ws[64:128, 0:6, :], in_=w3[:, 3:9, :])
        nc.sync.dma_start(out=bt[:], in_=bias[:])
        nc.vector.tensor_copy(wb[0:64, :, :], ws[0:64, :, :])
        nc.vector.tensor_copy(wb[64:128, 0:6, :], ws[64:128, 0:6, :])

        for n in range(NB):
            xf = xf_pool.tile([64, 64, 64], f32)
            nc.sync.dma_start(out=xf[:], in_=x[n])

            # padded bf16 image: lower[c, r, j] = img[c, r-1, j-1],
            # upper[c, r, j] = img[c, r, j-1]; zero borders everywhere else
            xb = xb_pool.tile([128, 66, 66], bf16)
            nc.gpsimd.memset(xb[0:64, 0:1, :], 0.0)
            nc.gpsimd.memset(xb[0:64, 65:66, :], 0.0)
            nc.gpsimd.memset(xb[0:64, 1:65, 0:1], 0.0)
            nc.gpsimd.memset(xb[0:64, 1:65, 65:66], 0.0)
            nc.gpsimd.memset(xb[64:128, 64:66, :], 0.0)
            if n % 2 == 0:
                nc.vector.tensor_copy(xb[0:64, 1:65, 1:65], xf[:])
            else:
                nc.scalar.copy(xb[0:64, 1:65, 1:65], xf[:])
            # upper half = lower shifted up one row (bf16 SBUF->SBUF)
            nc.sync.dma_start(out=xb[64:128, 0:64, :], in_=xb[0:64, 1:65, :])

            osb = ob_pool.tile([128, 64, 64], f32)
            for blk in range(8):
                y0 = blk * 8
                P = ps_pool.tile([128, 8, 64], f32)
                # taps (0,b) + (1,b) as K=128 vertical pairs
                for k, b in enumerate((0, 1, 2)):
                    nc.tensor.matmul(
                        P[:, :, :],
                        wb[:, b, :],
                        xb[:, y0:y0 + 8, b:b + 64],
                        start=(k == 0), stop=False,
                    )
                # taps (2,b) as K=64 on the row-shifted upper half
                for k, b in enumerate((0, 1, 2)):
                    nc.tensor.matmul(
                        P[:, :, :],
                        wb[64:128, 3 + b, :],
                        xb[64:128, y0 + 1:y0 + 9, b:b + 64],
                        start=False, stop=(k == 2),
                    )
                if blk % 2 == 0:
                    nc.vector.tensor_scalar_add(osb[:, y0:y0 + 8, :], P[:, :, :], bt[:])
                else:
                    nc.scalar.add(osb[:, y0:y0 + 8, :], P[:, :, :], bt[:])

            nc.sync.dma_start(out=out[n], in_=osb[:])

    nc.finalize()
    return nc


_NC = None


def _get_nc():
    global _NC
    if _NC is None:
        _NC = build_nc()
    return _NC


def kernel(**inputs) -> np.ndarray:
    x = np.ascontiguousarray(np.asarray(inputs["input"], dtype=np.float32))
    w = np.ascontiguousarray(np.asarray(inputs["weight"], dtype=np.float32))
    b = np.ascontiguousarray(
        np.asarray(inputs["bias"], dtype=np.float32).reshape(128, 1))
    nc = _get_nc()
    in_maps = [
        {"x": x[c * NB:(c + 1) * NB], "w": w, "b": b} for c in range(N_CORES)
    ]
    res = run_bass_kernel_spmd(nc, in_maps, list(range(N_CORES)))
    return np.concatenate([r["out"] for r in res.results], axis=0)


# revision 5
# speedup vs baseline: 1.4243x; 1.2192x over previous
"""Trainium2 Bass kernel for nn_Conv2d_20590073217670.

Conv2d: input [32,64,64,64] (NCHW), weight [576,128] (unfold layout:
row = ci*9 + a*3 + b for tap (a,b)), bias [1,128,1,1], stride 1, pad 1.
Output [32,128,64,64].

Strategy: data-parallel over batch — 4 images per NeuronCore, 8 cores.
Per image, implicit GEMM in bf16.  The image is cast fp32->bf16 into a
zero-padded [128, 66, 66] SBUF tile: partitions 0:64 hold
img[c, r-1, j-1] (zero border on every side), partitions 64:128 hold
img[c, r, j-1] (the same data one row up, built by a second cast on
the GpSimd engine in parallel with the DVE cast — no SBUF->SBUF DMA).
Per 8-row output block: 3 K=128 matmuls (vertical tap pairs
(0,b)+(1,b) across the two partition halves) plus 3 K=64 matmuls
(taps (2,b) from the lower half at +2 row offset) = 6 passes, each a
uniform full [8, 64] PSUM tile — border taps read the zero padding.
Two blocks are interleaved matmul-by-matmul so consecutive matmuls
target different PSUM banks and pipeline through the PE array instead
of serializing on same-bank accumulation.  PSUM eviction (fused bias
add) alternates DVE/Act per block.  DMA queues are split: inputs on
the SP HWDGE ring, output halves alternate between the Act and SP
rings, issued per half-image so stores overlap compute.
"""
import sys

for _p in ("/opt/trn_rl_repo", "/root/.axon_site/_ro/trn_rl_repo"):
    if _p not in sys.path:
        sys.path.append(_p)

import numpy as np
from contextlib import ExitStack

import concourse.bacc as bacc
import concourse.tile as tile
from concourse import mybir
from concourse.bass_utils import run_bass_kernel_spmd

f32 = mybir.dt.float32
bf16 = mybir.dt.bfloat16

N_CORES = 8
NB = 4  # images per core


def build_nc():
    nc = bacc.Bacc()
    x = nc.declare_dram_parameter("x", [NB, 64, 64, 64], f32, isOutput=False)
    w = nc.declare_dram_parameter("w", [576, 128], f32, isOutput=False)
    bias = nc.declare_dram_parameter("b", [128, 1], f32, isOutput=False)
    out = nc.declare_dram_parameter("out", [NB, 128, 64, 64], f32, isOutput=True)

    with tile.TileContext(nc) as tc, ExitStack() as ctx:
        const = ctx.enter_context(tc.tile_pool(name="const", bufs=1))
        xf_pool = ctx.enter_context(tc.tile_pool(name="xf", bufs=4))
        ob_pool = ctx.enter_context(tc.tile_pool(name="ob", bufs=2))
        ps_pool = ctx.enter_context(tc.tile_pool(name="ps", bufs=4, space="PSUM"))

        # ---- weights [128, 9, 128] bf16: partition p<64 holds channel p's
        # taps 0..8; partition 64+ci holds channel ci's taps 3..8 at slots
        # 0..5 (tap axis pre-shifted by -3).  wb[:, b, :] pairs taps (0,b)
        # lower / (1,b) upper; wb[0:64, 6+b, :] is tap (2,b).
        w3 = w[:].rearrange("(c t) m -> c t m", t=9)
        ws = const.tile([128, 9, 128], f32)
        wb = const.tile([128, 9, 128], bf16)
        bt = const.tile([128, 1], f32)
        nc.sync.dma_start(out=ws[0:64, :, :], in_=w3)
        nc.sync.dma_start(out=ws[64:128, 0:6, :], in_=w3[:, 3:9, :])
        nc.sync.dma_start(out=bt[:], in_=bias[:])
        nc.vector.tensor_copy(wb[0:64, :, :], ws[0:64, :, :])
        nc.vector.tensor_copy(wb[64:128, 0:6, :], ws[64:128, 0:6, :])

        # ---- two persistent padded image tiles, manually double-buffered.
        # Interiors are rewritten every image; borders are zeroed once here.
        xb0 = const.tile([128, 66, 66], bf16)
        xb1 = const.tile([128, 66, 66], bf16)
        xbs = [xb0, xb1]
        for xb in xbs:
            nc.gpsimd.memset(xb[0:64, 0:1, :], 0.0)
            nc.gpsimd.memset(xb[0:64, 65:66, :], 0.0)
            nc.gpsimd.memset(xb[0:64, :, 0:1], 0.0)
            nc.gpsimd.memset(xb[0:64, :, 65:66], 0.0)
            nc.gpsimd.memset(xb[64:128, 0:64, 0:1], 0.0)
            nc.gpsimd.memset(xb[64:128, 0:64, 65:66], 0.0)

        for n in range(NB):
            xf = xf_pool.tile([64, 64, 64], f32)
            nc.sync.dma_start(out=xf[:], in_=x[n])

            xb = xbs[n % 2]
            # lower half on DVE, upper (one-row-up) half on GpSimd, parallel
            nc.vector.tensor_copy(xb[0:64, 1:65, 1:65], xf[:])
            nc.gpsimd.tensor_copy(xb[64:128, 0:64, 1:65], xf[:])

            osb = ob_pool.tile([128, 64, 64], f32)
            for grp in range(4):
                y0a = grp * 16
                y0b = y0a + 8
                PA = ps_pool.tile([128, 8, 64], f32)
                PB = ps_pool.tile([128, 8, 64], f32)
                for p in range(6):
                    st, sp = (p == 0), (p == 5)
                    if p < 3:
                        b = p
                        for P, y0 in ((PA, y0a), (PB, y0b)):
                            nc.tensor.matmul(
                                P[:, :, :], wb[:, b, :],
                                xb[:, y0:y0 + 8, b:b + 64],
                                start=st, stop=sp,
                            )
                    else:
                        b = p - 3
                        for P, y0 in ((PA, y0a), (PB, y0b)):
                            nc.tensor.matmul(
                                P[:, :, :], wb[0:64, 6 + b, :],
                                xb[0:64, y0 + 2:y0 + 10, b:b + 64],
                                start=st, stop=sp,
                            )
                nc.vector.tensor_scalar_add(osb[:, y0a:y0a + 8, :], PA[:, :, :], bt[:])
                nc.scalar.add(osb[:, y0b:y0b + 8, :], PB[:, :, :], bt[:])
                if grp == 1:
                    nc.scalar.dma_start(out=out[n][:, 0:32, :], in_=osb[:, 0:32, :])
                elif grp == 3:
                    nc.sync.dma_start(out=out[n][:, 32:64, :], in_=osb[:, 32:64, :])

    nc.finalize()
    return nc


_NC = None


def _get_nc():
    global _NC
    if _NC is None:
        _NC = build_nc()
    return _NC


def kernel(**inputs) -> np.ndarray:
    x = np.ascontiguousarray(np.asarray(inputs["input"], dtype=np.float32))
    w = np.ascontiguousarray(np.asarray(inputs["weight"], dtype=np.float32))
    b = np.ascontiguousarray(
        np.asarray(inputs["bias"], dtype=np.float32).reshape(128, 1))
    nc = _get_nc()
    in_maps = [
        {"x": x[c * NB:(c + 1) * NB], "w": w, "b": b} for c in range(N_CORES)
    ]
    res = run_bass_kernel_spmd(nc, in_maps, list(range(N_CORES)))
    return np.concatenate([r["out"] for r in res.results], axis=0)


# revision 6
# speedup vs baseline: 1.7701x; 1.2427x over previous
"""Trainium2 Bass kernel for nn_Conv2d_20590073217670.

Conv2d: input [32,64,64,64] (NCHW), weight [576,128] (unfold layout:
row = ci*9 + a*3 + b for tap (a,b)), bias [1,128,1,1], stride 1, pad 1.
Output [32,128,64,64].

Strategy: data-parallel over batch — 4 images per NeuronCore, 8 cores.
Per image, implicit GEMM in bf16.  The image is cast fp32->bf16 into a
zero-padded [128, 66, 66] SBUF tile: partitions 0:64 hold
img[c, r-1, j-1] (zero border on every side), partitions 64:128 hold
img[c, r, j-1] (the same data one row up, built by a second cast on
the GpSimd engine in parallel with the DVE cast — no SBUF->SBUF DMA).
Per 8-row output block: 3 K=128 matmuls (vertical tap pairs
(0,b)+(1,b) across the two partition halves) plus 3 K=64 matmuls
(taps (2,b) from the lower half at +2 row offset) = 6 passes, each a
uniform full [8, 64] PSUM tile — border taps read the zero padding.
Two blocks are interleaved matmul-by-matmul so consecutive matmuls
target different PSUM banks and pipeline through the PE array instead
of serializing on same-bank accumulation.  PSUM eviction (fused bias
add) alternates DVE/Act per block.  DMA queues are split: inputs on
the SP HWDGE ring, output halves alternate between the Act and SP
rings, issued per half-image so stores overlap compute.
"""
import sys

for _p in ("/opt/trn_rl_repo", "/root/.axon_site/_ro/trn_rl_repo"):
    if _p not in sys.path:
        sys.path.append(_p)

import numpy as np
from contextlib import ExitStack

import concourse.bacc as bacc
import concourse.tile as tile
from concourse import mybir
from concourse.bass_utils import run_bass_kernel_spmd

f32 = mybir.dt.float32
bf16 = mybir.dt.bfloat16

N_CORES = 8
NB = 4  # images per core


def build_nc():
    nc = bacc.Bacc()
    x = nc.declare_dram_parameter("x", [NB, 64, 64, 64], f32, isOutput=False)
    w = nc.declare_dram_parameter("w", [576, 128], f32, isOutput=False)
    bias = nc.declare_dram_parameter("b", [128, 1], f32, isOutput=False)
    out = nc.declare_dram_parameter("out", [NB, 128, 64, 64], f32, isOutput=True)

    with tile.TileContext(nc) as tc, ExitStack() as ctx:
        const = ctx.enter_context(tc.tile_pool(name="const", bufs=1))
        xf_pool = ctx.enter_context(tc.tile_pool(name="xf", bufs=4))
        ob_pool = ctx.enter_context(tc.tile_pool(name="ob", bufs=2))
        ps_pool = ctx.enter_context(tc.tile_pool(name="ps", bufs=4, space="PSUM"))

        # ---- weights [128, 9, 128] bf16: partition p<64 holds channel p's
        # taps 0..8; partition 64+ci holds channel ci's taps 3..8 at slots
        # 0..5 (tap axis pre-shifted by -3).  wb[:, b, :] pairs taps (0,b)
        # lower / (1,b) upper; wb[0:64, 6+b, :] is tap (2,b).
        w3 = w[:].rearrange("(c t) m -> c t m", t=9)
        ws = const.tile([128, 9, 128], f32)
        wb = const.tile([128, 9, 128], bf16)
        bt = const.tile([128, 1], f32)
        nc.sync.dma_start(out=ws[0:64, :, :], in_=w3)
        nc.sync.dma_start(out=ws[64:128, 0:6, :], in_=w3[:, 3:9, :])
        nc.sync.dma_start(out=bt[:], in_=bias[:])
        nc.vector.tensor_copy(wb[0:64, :, :], ws[0:64, :, :])
        nc.vector.tensor_copy(wb[64:128, 0:6, :], ws[64:128, 0:6, :])

        # ---- two persistent padded image tiles, manually double-buffered.
        # Interiors are rewritten every image; borders are zeroed once here.
        xb0 = const.tile([128, 66, 66], bf16)
        xb1 = const.tile([128, 66, 66], bf16)
        xbs = [xb0, xb1]
        for xb in xbs:
            nc.gpsimd.memset(xb[0:64, 0:1, :], 0.0)
            nc.gpsimd.memset(xb[0:64, 65:66, :], 0.0)
            nc.gpsimd.memset(xb[0:64, :, 0:1], 0.0)
            nc.gpsimd.memset(xb[0:64, :, 65:66], 0.0)
            nc.gpsimd.memset(xb[64:128, 0:64, 0:1], 0.0)
            nc.gpsimd.memset(xb[64:128, 0:64, 65:66], 0.0)

        # input/cast/shift chunking: rows [0:34) cover matmul groups 0-1
        # (they read padded lower rows <=33 / upper rows <=31), [34:64) the rest
        CH = ((0, 34), (34, 64))

        for n in range(NB):
            xf = xf_pool.tile([64, 64, 64], f32)
            for r0, r1 in CH:
                nc.sync.dma_start(out=xf[:, r0:r1, :], in_=x[n][:, r0:r1, :])

            xb = xbs[n % 2]
            for r0, r1 in CH:
                # lower half: fp32 -> bf16 cast on DVE
                nc.vector.tensor_copy(xb[0:64, 1 + r0:1 + r1, 1:65], xf[:, r0:r1, :])
                # upper half (one row up) = bf16 copy of the lower rows,
                # SBUF->SBUF DMA on the Act HWDGE ring
                nc.scalar.dma_start(
                    out=xb[64:128, r0:r1, :], in_=xb[0:64, 1 + r0:1 + r1, :])

            osb = ob_pool.tile([128, 64, 64], f32)
            for grp in range(4):
                y0a = grp * 16
                y0b = y0a + 8
                PA = ps_pool.tile([128, 8, 64], f32)
                PB = ps_pool.tile([128, 8, 64], f32)
                for p in range(6):
                    st, sp = (p == 0), (p == 5)
                    if p < 3:
                        b = p
                        for P, y0 in ((PA, y0a), (PB, y0b)):
                            nc.tensor.matmul(
                                P[:, :, :], wb[:, b, :],
                                xb[:, y0:y0 + 8, b:b + 64],
                                start=st, stop=sp,
                            )
                    else:
                        b = p - 3
                        for P, y0 in ((PA, y0a), (PB, y0b)):
                            nc.tensor.matmul(
                                P[:, :, :], wb[0:64, 6 + b, :],
                                xb[0:64, y0 + 2:y0 + 10, b:b + 64],
                                start=st, stop=sp,
                            )
                nc.vector.tensor_scalar_add(osb[:, y0a:y0a + 8, :], PA[:, :, :], bt[:])
                nc.scalar.add(osb[:, y0b:y0b + 8, :], PB[:, :, :], bt[:])
                # stream this group's 16 output rows out, alternating rings
                dma_eng = nc.scalar if grp % 2 == 0 else nc.sync
                dma_eng.dma_start(
                    out=out[n][:, y0a:y0a + 16, :], in_=osb[:, y0a:y0a + 16, :])

    nc.finalize()
    return nc


_NC = None


def _get_nc():
    global _NC
    if _NC is None:
        _NC = build_nc()
    return _NC


def kernel(**inputs) -> np.ndarray:
    x = np.ascontiguousarray(np.asarray(inputs["input"], dtype=np.float32))
    w = np.ascontiguousarray(np.asarray(inputs["weight"], dtype=np.float32))
    b = np.ascontiguousarray(
        np.asarray(inputs["bias"], dtype=np.float32).reshape(128, 1))
    nc = _get_nc()
    in_maps = [
        {"x": x[c * NB:(c + 1) * NB], "w": w, "b": b} for c in range(N_CORES)
    ]
    res = run_bass_kernel_spmd(nc, in_maps, list(range(N_CORES)))
    return np.concatenate([r["out"] for r in res.results], axis=0)


# revision 8
# speedup vs baseline: 1.9236x; 1.0868x over previous
"""Trainium2 Bass kernel for nn_Conv2d_20590073217670.

Conv2d: input [32,64,64,64] (NCHW), weight [576,128] (unfold layout:
row = ci*9 + a*3 + b for tap (a,b)), bias [1,128,1,1], stride 1, pad 1.
Output [32,128,64,64].

Strategy: data-parallel over batch — 4 images per NeuronCore, 8 cores.
Per image, implicit GEMM in bf16.  The image is cast fp32->bf16 into a
zero-padded [128, 66, 66] SBUF tile: partitions 0:64 hold
img[c, r-1, j-1] (zero border on every side), partitions 64:128 hold
img[c, r, j-1] (the same data one row up, built by a second cast on
the GpSimd engine in parallel with the DVE cast — no SBUF->SBUF DMA).
Per 8-row output block: 3 K=128 matmuls (vertical tap pairs
(0,b)+(1,b) across the two partition halves) plus 3 K=64 matmuls
(taps (2,b) from the lower half at +2 row offset) = 6 passes, each a
uniform full [8, 64] PSUM tile — border taps read the zero padding.
Two blocks are interleaved matmul-by-matmul so consecutive matmuls
target different PSUM banks and pipeline through the PE array instead
of serializing on same-bank accumulation.  PSUM eviction (fused bias
add) alternates DVE/Act per block.  DMA queues are split: inputs on
the SP HWDGE ring, output halves alternate between the Act and SP
rings, issued per half-image so stores overlap compute.
"""
import sys

for _p in ("/opt/trn_rl_repo", "/root/.axon_site/_ro/trn_rl_repo"):
    if _p not in sys.path:
        sys.path.append(_p)

import numpy as np
from contextlib import ExitStack

import concourse.bacc as bacc
import concourse.tile as tile
from concourse import mybir
from concourse.bass_utils import run_bass_kernel_spmd

f32 = mybir.dt.float32
bf16 = mybir.dt.bfloat16

N_CORES = 8
NB = 4  # images per core


def build_nc():
    nc = bacc.Bacc()
    x = nc.declare_dram_parameter("x", [NB, 64, 64, 64], f32, isOutput=False)
    w = nc.declare_dram_parameter("w", [576, 128], f32, isOutput=False)
    bias = nc.declare_dram_parameter("b", [128, 1], f32, isOutput=False)
    out = nc.declare_dram_parameter("out", [NB, 128, 64, 64], f32, isOutput=True)

    with tile.TileContext(nc) as tc, ExitStack() as ctx:
        const = ctx.enter_context(tc.tile_pool(name="const", bufs=1))
        xf_pool = ctx.enter_context(tc.tile_pool(name="xf", bufs=2))
        ob_pool = ctx.enter_context(tc.tile_pool(name="ob", bufs=2))
        ps_pool = ctx.enter_context(tc.tile_pool(name="ps", bufs=2, space="PSUM"))

        # ---- weights [128, 9, 128] bf16: partition p<64 holds channel p's
        # taps 0..8; partition 64+ci holds channel ci's taps 3..8 at slots
        # 0..5 (tap axis pre-shifted by -3).  wb[:, b, :] pairs taps (0,b)
        # lower / (1,b) upper; wb[0:64, 6+b, :] is tap (2,b).
        w3 = w[:].rearrange("(c t) m -> c t m", t=9)
        ws = const.tile([128, 9, 128], f32)
        wb = const.tile([128, 9, 128], bf16)
        bt = const.tile([128, 1], f32)
        nc.sync.dma_start(out=ws[0:64, :, :], in_=w3)
        nc.sync.dma_start(out=ws[64:128, 0:6, :], in_=w3[:, 3:9, :])
        nc.sync.dma_start(out=bt[:], in_=bias[:])
        nc.vector.tensor_copy(wb[0:64, :, :], ws[0:64, :, :])
        nc.vector.tensor_copy(wb[64:128, 0:6, :], ws[64:128, 0:6, :])

        # ---- two persistent padded image tiles, manually double-buffered.
        # Interiors are rewritten every image; borders are zeroed once here.
        xb0 = const.tile([128, 66, 66], bf16)
        xb1 = const.tile([128, 66, 66], bf16)
        xbs = [xb0, xb1]
        for xb in xbs:
            nc.gpsimd.memset(xb[0:64, 0:1, :], 0.0)
            nc.gpsimd.memset(xb[0:64, 65:66, :], 0.0)
            nc.gpsimd.memset(xb[0:64, :, 0:1], 0.0)
            nc.gpsimd.memset(xb[0:64, :, 65:66], 0.0)
            nc.gpsimd.memset(xb[64:128, 0:64, 0:1], 0.0)
            nc.gpsimd.memset(xb[64:128, 0:64, 65:66], 0.0)

        # input/cast/shift chunking: rows [0:34) cover matmul groups 0-1
        # (they read padded lower rows <=33 / upper rows <=31), [34:64) the rest
        CH = ((0, 34), (34, 64))

        for n in range(NB):
            xf = xf_pool.tile([64, 64, 64], f32)
            for r0, r1 in CH:
                nc.sync.dma_start(out=xf[:, r0:r1, :], in_=x[n][:, r0:r1, :])

            xb = xbs[n % 2]
            for r0, r1 in CH:
                # lower half: fp32 -> bf16 cast on DVE
                nc.vector.tensor_copy(xb[0:64, 1 + r0:1 + r1, 1:65], xf[:, r0:r1, :])
                # upper half (one row up) = bf16 copy of the lower rows,
                # SBUF->SBUF DMA on the otherwise-idle GpSimd SWDGE queue
                nc.gpsimd.dma_start(
                    out=xb[64:128, r0:r1, :], in_=xb[0:64, 1 + r0:1 + r1, :])

            osb = ob_pool.tile([128, 64, 64], f32)
            for half in range(2):
                # pass-major over 4 blocks: consecutive matmuls rotate over 4
                # PSUM banks, pipelining the PE and reusing each weight 4x
                P0 = ps_pool.tile([128, 8, 64], f32)
                P1 = ps_pool.tile([128, 8, 64], f32)
                P2 = ps_pool.tile([128, 8, 64], f32)
                P3 = ps_pool.tile([128, 8, 64], f32)
                Ps = (P0, P1, P2, P3)
                ys = [half * 32 + q * 8 for q in range(4)]
                for p in range(6):
                    st, sp = (p == 0), (p == 5)
                    if p < 3:
                        b = p
                        for P, y0 in zip(Ps, ys):
                            nc.tensor.matmul(
                                P[:, :, :], wb[:, b, :],
                                xb[:, y0:y0 + 8, b:b + 64],
                                start=st, stop=sp,
                            )
                    else:
                        b = p - 3
                        for P, y0 in zip(Ps, ys):
                            nc.tensor.matmul(
                                P[:, :, :], wb[0:64, 6 + b, :],
                                xb[0:64, y0 + 2:y0 + 10, b:b + 64],
                                start=st, stop=sp,
                            )
                for q, (P, y0) in enumerate(zip(Ps, ys)):
                    if q % 2 == 0:
                        nc.vector.tensor_scalar_add(osb[:, y0:y0 + 8, :], P[:, :, :], bt[:])
                    else:
                        nc.scalar.add(osb[:, y0:y0 + 8, :], P[:, :, :], bt[:])
                    if q % 2 == 1:
                        # stream each evicted 16-row group out, alternating rings
                        dma_eng = nc.scalar if (half * 2 + q // 2) % 2 == 0 else nc.sync
                        dma_eng.dma_start(
                            out=out[n][:, y0 - 8:y0 + 8, :],
                            in_=osb[:, y0 - 8:y0 + 8, :])

    nc.finalize()
    return nc


_NC = None


def _get_nc():
    global _NC
    if _NC is None:
        _NC = build_nc()
    return _NC


def kernel(**inputs) -> np.ndarray:
    x = np.ascontiguousarray(np.asarray(inputs["input"], dtype=np.float32))
    w = np.ascontiguousarray(np.asarray(inputs["weight"], dtype=np.float32))
    b = np.ascontiguousarray(
        np.asarray(inputs["bias"], dtype=np.float32).reshape(128, 1))
    nc = _get_nc()
    in_maps = [
        {"x": x[c * NB:(c + 1) * NB], "w": w, "b": b} for c in range(N_CORES)
    ]
    res = run_bass_kernel_spmd(nc, in_maps, list(range(N_CORES)))
    return np.concatenate([r["out"] for r in res.results], axis=0)


# revision 9
# speedup vs baseline: 1.9772x; 1.0278x over previous
"""Trainium2 Bass kernel for nn_Conv2d_20590073217670.

Conv2d: input [32,64,64,64] (NCHW), weight [576,128] (unfold layout:
row = ci*9 + a*3 + b for tap (a,b)), bias [1,128,1,1], stride 1, pad 1.
Output [32,128,64,64].

Strategy: data-parallel over batch — 4 images per NeuronCore, 8 cores.
Per image, implicit GEMM in bf16 with two padded [128, 66, 66] SBUF
layouts, all built by direct fp32->bf16 casts from the staged input
(DVE casts the lower halves, Act the upper halves, in parallel):
  xb: parts 0:64  = img[c, r-1, j-1]   (zero border all sides)
      parts 64:128 = img[c, r,   j-1]  (one row up)
  xc: parts 0:64  = img[c, r-1, j-1]
      parts 64:128 = img[c, r-1, j  ]  (one col left)
Per 8-row output block, 5 matmul passes, each a uniform full [8, 64]
PSUM tile (border taps read zero padding):
  3x K=128: vertical tap pairs (0,b)+(1,b) from xb        (b = 0,1,2)
  1x K=128: horizontal tap pair (2,0)+(2,1) from xc at +2 rows
  1x K=64 : tap (2,2) from xb lower at +2 rows, col 2
Four blocks are processed pass-major so consecutive matmuls rotate
over 4 PSUM banks and pipeline through the PE array (same-bank
accumulation serializes at ~465ns/matmul; rotated it runs at ~220ns).
PSUM eviction (fused bias add) alternates DVE/Act per bank; output
streams out per 16 rows alternating between the two HWDGE rings.
Inputs are chunked in three so the first matmul can start early.
"""
import sys

for _p in ("/opt/trn_rl_repo", "/root/.axon_site/_ro/trn_rl_repo"):
    if _p not in sys.path:
        sys.path.append(_p)

import numpy as np
from contextlib import ExitStack

import concourse.bacc as bacc
import concourse.tile as tile
from concourse import mybir
from concourse.bass_utils import run_bass_kernel_spmd

f32 = mybir.dt.float32
bf16 = mybir.dt.bfloat16

N_CORES = 8
NB = 4  # images per core


def build_nc():
    nc = bacc.Bacc()
    x = nc.declare_dram_parameter("x", [NB, 64, 64, 64], f32, isOutput=False)
    w = nc.declare_dram_parameter("w", [576, 128], f32, isOutput=False)
    bias = nc.declare_dram_parameter("b", [128, 1], f32, isOutput=False)
    out = nc.declare_dram_parameter("out", [NB, 128, 64, 64], f32, isOutput=True)

    with tile.TileContext(nc) as tc, ExitStack() as ctx:
        const = ctx.enter_context(tc.tile_pool(name="const", bufs=1))
        xf_pool = ctx.enter_context(tc.tile_pool(name="xf", bufs=2))
        ob_pool = ctx.enter_context(tc.tile_pool(name="ob", bufs=2))
        ps_pool = ctx.enter_context(tc.tile_pool(name="ps", bufs=2, space="PSUM"))

        # ---- weights.  wb [128, 9, 128] bf16: partition p<64 holds channel
        # p's taps 0..8; partition 64+ci holds taps 3..8 at slots 0..5, so
        # wb[:, b, :] pairs taps (0,b) lower / (1,b) upper and wb[0:64, 8, :]
        # is tap (2,2).  wc [128, 128] pairs taps (2,0) lower / (2,1) upper.
        w3 = w[:].rearrange("(c t) m -> c t m", t=9)
        ws = const.tile([128, 9, 128], f32)
        wsc = const.tile([128, 128], f32)
        wb = const.tile([128, 9, 128], bf16)
        wc = const.tile([128, 128], bf16)
        bt = const.tile([128, 1], f32)
        nc.scalar.dma_start(out=ws[0:64, :, :], in_=w3)
        nc.scalar.dma_start(out=ws[64:128, 0:6, :], in_=w3[:, 3:9, :])
        nc.scalar.dma_start(out=wsc[0:64, :], in_=w3[:, 6, :])
        nc.scalar.dma_start(out=wsc[64:128, :], in_=w3[:, 7, :])
        nc.scalar.dma_start(out=bt[:], in_=bias[:])
        nc.vector.tensor_copy(wb[0:64, :, :], ws[0:64, :, :])
        nc.vector.tensor_copy(wb[64:128, 0:6, :], ws[64:128, 0:6, :])
        nc.vector.tensor_copy(wc[:, :], wsc[:, :])

        # ---- two persistent padded image tile sets, manually double-
        # buffered; interiors are rewritten every image, borders zeroed once
        xb0 = const.tile([128, 66, 66], bf16)
        xb1 = const.tile([128, 66, 66], bf16)
        xc0 = const.tile([128, 66, 66], bf16)
        xc1 = const.tile([128, 66, 66], bf16)
        for xb in (xb0, xb1):
            nc.gpsimd.memset(xb[0:64, 0:1, :], 0.0)
            nc.gpsimd.memset(xb[0:64, 65:66, :], 0.0)
            nc.gpsimd.memset(xb[0:64, :, 0:1], 0.0)
            nc.gpsimd.memset(xb[0:64, :, 65:66], 0.0)
            nc.gpsimd.memset(xb[64:128, 0:64, 0:1], 0.0)
            nc.gpsimd.memset(xb[64:128, 0:64, 65:66], 0.0)
        for xc in (xc0, xc1):
            nc.gpsimd.memset(xc[0:64, 65:66, :], 0.0)
            nc.gpsimd.memset(xc[0:64, :, 0:1], 0.0)
            nc.gpsimd.memset(xc[64:128, 65:66, 0:64], 0.0)

        # input/cast chunk boundaries (image rows); chunks A+B cover the
        # first 4-block half's reads (padded rows <= 34), C the rest
        CH = ((0, 18), (18, 34), (34, 64))

        for n in range(NB):
            xf = xf_pool.tile([64, 64, 64], f32)
            for r0, r1 in CH:
                nc.sync.dma_start(out=xf[:, r0:r1, :], in_=x[n][:, r0:r1, :])

            xb = (xb0, xb1)[n % 2]
            xc = (xc0, xc1)[n % 2]
            for r0, r1 in CH:
                nc.vector.tensor_copy(xb[0:64, 1 + r0:1 + r1, 1:65], xf[:, r0:r1, :])
                nc.scalar.copy(xb[64:128, r0:r1, 1:65], xf[:, r0:r1, :])
                nc.vector.tensor_copy(xc[0:64, 1 + r0:1 + r1, 1:65], xf[:, r0:r1, :])
                nc.scalar.copy(xc[64:128, 1 + r0:1 + r1, 0:64], xf[:, r0:r1, :])

            osb = ob_pool.tile([128, 64, 64], f32)
            for half in range(2):
                # pass-major over 4 blocks: consecutive matmuls rotate over 4
                # PSUM banks, pipelining the PE and reusing each weight 4x
                P0 = ps_pool.tile([128, 8, 64], f32)
                P1 = ps_pool.tile([128, 8, 64], f32)
                P2 = ps_pool.tile([128, 8, 64], f32)
                P3 = ps_pool.tile([128, 8, 64], f32)
                Ps = (P0, P1, P2, P3)
                ys = [half * 32 + q * 8 for q in range(4)]
                for p in range(5):
                    st, sp = (p == 0), (p == 4)
                    for P, y0 in zip(Ps, ys):
                        if p < 3:
                            nc.tensor.matmul(
                                P[:, :, :], wb[:, p, :],
                                xb[:, y0:y0 + 8, p:p + 64],
                                start=st, stop=sp,
                            )
                        elif p == 3:
                            nc.tensor.matmul(
                                P[:, :, :], wc[:, :],
                                xc[:, y0 + 2:y0 + 10, 0:64],
                                start=st, stop=sp,
                            )
                        else:
                            nc.tensor.matmul(
                                P[:, :, :], wb[0:64, 8, :],
                                xb[0:64, y0 + 2:y0 + 10, 2:66],
                                start=st, stop=sp,
                            )
                for q, (P, y0) in enumerate(zip(Ps, ys)):
                    if q % 2 == 0:
                        nc.vector.tensor_scalar_add(osb[:, y0:y0 + 8, :], P[:, :, :], bt[:])
                    else:
                        nc.scalar.add(osb[:, y0:y0 + 8, :], P[:, :, :], bt[:])
                    if n == NB - 1:
                        # last image: stream each 8-row bank out immediately
                        dma_eng = nc.scalar if q % 2 == 0 else nc.sync
                        dma_eng.dma_start(
                            out=out[n][:, y0:y0 + 8, :], in_=osb[:, y0:y0 + 8, :])
                    elif q % 2 == 1:
                        dma_eng = nc.scalar if (half * 2 + q // 2) % 2 == 0 else nc.sync
                        dma_eng.dma_start(
                            out=out[n][:, y0 - 8:y0 + 8, :],
                            in_=osb[:, y0 - 8:y0 + 8, :])

    nc.finalize()
    return nc


_NC = None


def _get_nc():
    global _NC
    if _NC is None:
        _NC = build_nc()
    return _NC


def kernel(**inputs) -> np.ndarray:
    x = np.ascontiguousarray(np.asarray(inputs["input"], dtype=np.float32))
    w = np.ascontiguousarray(np.asarray(inputs["weight"], dtype=np.float32))
    b = np.ascontiguousarray(
        np.asarray(inputs["bias"], dtype=np.float32).reshape(128, 1))
    nc = _get_nc()
    in_maps = [
        {"x": x[c * NB:(c + 1) * NB], "w": w, "b": b} for c in range(N_CORES)
    ]
    res = run_bass_kernel_spmd(nc, in_maps, list(range(N_CORES)))
    return np.concatenate([r["out"] for r in res.results], axis=0)
